# revision 34
# baseline (speedup 1.0000x reference)
"""Distributed Trainium2 Bass kernel for causal GQA attention block.

Problem (hardcoded): x [4, 2048, 1024] f32; wq [1024, 1024]; wk/wv [1024, 256];
wo [1024, 1024]. 16 q-heads, 4 kv-heads, head_dim 64, rms-norm on q/k (no
weight), rope (base 10000), q gain 1.5, causal SDPA, out-proj.

Sharding over 8 cores: core i -> batch b = i//2, head-half p = i%2
(q-heads 8p..8p+7, kv-heads 2p, 2p+1 -- KV groups intact). Each core computes
its 8 heads' attention output O^T (feature-major), then a PARTIAL out-proj
over its own 512 contraction dims against wo[own rows, all 1024 cols]; a
pairwise ReduceScatter(add) finishes the projection and leaves each core
with its disjoint 512-column output slice.

On-chip layouts are feature-major ("transposed"): X^T, Q^T, K^T so the PE
contracts over partitions; V is token-major with a ones column appended so the
PV matmul also produces softmax row-sums (normalization happens on O^T).
"""
import sys

sys.path.insert(0, "/opt/trn_rl_repo")

import numpy as np
import ml_dtypes

import concourse.bacc as bacc
import concourse.mybir as mybir
import concourse.tile as tile
from concourse.bass_utils import run_bass_kernel_spmd

F32 = mybir.dt.float32
BF16 = mybir.dt.bfloat16
AF = mybir.ActivationFunctionType

N = 2048          # tokens
C = 1024          # model dim
DQ = 512          # local q out-features (8 heads x 64)
DKV = 128         # local kv out-features (2 kv heads x 64)
D = 64            # head dim
NCC = C // 128    # 8 contraction chunks
NQT = 4           # q tiles of 512
NTC = N // 128    # 16 token chunks
QK_GAIN = 1.5
ROPE_BASE = 10000.0
EXP_SCALE = QK_GAIN / np.sqrt(D).item()  # folded gain * 1/sqrt(D) = 0.1875
EPS = float(np.finfo(np.float32).eps)


def _host_tables():
    inv_freq = (1.0 / (ROPE_BASE ** (np.arange(0, D, 2, dtype=np.float64) / D)))  # [32]
    t = np.arange(N, dtype=np.float64)
    ang = np.outer(inv_freq, t)  # [32, N]
    cos32 = np.cos(ang)
    sin32 = np.sin(ang)
    cosT = np.tile(cos32, (4, 1)).astype(np.float32)  # [128, N]
    sinTs = np.concatenate([-sin32, sin32, -sin32, sin32], axis=0).astype(np.float32)
    q = np.arange(128)
    trimask = (q[None, :] >= q[:, None]).astype(np.float32)  # keep q >= k
    # block-diagonal ones: reduce-over-64-feats that lands pre-broadcast
    blkones = np.zeros((128, 128), np.float32)
    blkones[0:64, 0:64] = 1.0
    blkones[64:128, 64:128] = 1.0
    ident = np.eye(128, dtype=np.float32)
    bf = ml_dtypes.bfloat16
    return {
        "cosT": cosT.astype(bf),
        "sinTs": sinTs.astype(bf),
        "trimask": trimask.astype(bf),
        "blkones": blkones.astype(bf),
        "ident": ident.astype(bf),
    }


def build():
    nc = bacc.Bacc(None, target_bir_lowering=False, num_devices=8)

    # x arrives host-pre-transposed: [C, N] feature-major
    x_ext = nc.declare_dram_parameter("x", [C, N], BF16, isOutput=False)
    wq_ext = nc.declare_dram_parameter("wq", [C, DQ], BF16, isOutput=False)
    wk_ext = nc.declare_dram_parameter("wk", [C, DKV], BF16, isOutput=False)
    wv_ext = nc.declare_dram_parameter("wv", [C, DKV], BF16, isOutput=False)
    # wo here is the core's OWN 512 contraction rows x all 1024 out-features.
    wo_ext = nc.declare_dram_parameter("wo", [DQ, C], BF16, isOutput=False)
    out_ext = nc.declare_dram_parameter("out", [N, DQ], BF16, isOutput=True)

    tabs = _host_tables()
    cosT_d = nc.inline_tensor(tabs["cosT"], name="cosT_d")
    sinTs_d = nc.inline_tensor(tabs["sinTs"], name="sinTs_d")
    trimask_d = nc.inline_tensor(tabs["trimask"], name="trimask_d")
    blkones_d = nc.inline_tensor(tabs["blkones"], name="blkones_d")
    ident_d = nc.inline_tensor(tabs["ident"], name="ident_d")

    with tile.TileContext(nc) as tc:
        with (
            tc.tile_pool(name="dram", bufs=1, space="DRAM") as dram,
            tc.tile_pool(name="persist", bufs=1) as ps,
        ):
            # ---- persistent SBUF tensors ----
            xT = ps.tile([128, NCC, N], BF16, name="xT")          # X^T chunks
            wq_sb = ps.tile([128, NCC, DQ], BF16, name="wq_sb")
            wk_sb = ps.tile([128, NCC, DKV], BF16, name="wk_sb")
            wv_sb = ps.tile([128, NCC, DKV], BF16, name="wv_sb")
            wo_sb = ps.tile([128, 4, C], BF16, name="wo_sb")      # own rows x 1024
            cosT = ps.tile([128, N], BF16, name="cosT")
            sinTs = ps.tile([128, N], BF16, name="sinTs")
            trimask = ps.tile([128, 128], BF16, name="trimask")
            blkones = ps.tile([128, 128], BF16, name="blkones")
            ident = ps.tile([128, 128], BF16, name="ident")
            eps_sb = ps.tile([128, 1], F32, name="eps_sb")
            kTdA = ps.tile([128, N], BF16, name="kTdA")           # kv head A dup'd
            kTdB = ps.tile([128, N], BF16, name="kTdB")
            # [V_A(64)|1s(64)|V_B(64)|1s(64)] -> PV rows 64:128 = rowsum bcast
            v_sb = ps.tile([128, NTC, 256], BF16, name="v_sb")
            oT = ps.tile([128, 4, N], BF16, name="oT")            # own O^T (normed)

            # ---- phase A: stage inputs (host supplies bf16, x pre-transposed) ----
            # wk/wq first (first matmuls need them), x chunks split across queues
            nc.gpsimd.dma_start(out=wk_sb[:], in_=wk_ext.rearrange("(a p) j -> p a j", p=128))
            nc.gpsimd.dma_start(out=wv_sb[:], in_=wv_ext.rearrange("(a p) j -> p a j", p=128))
            nc.gpsimd.dma_start(out=wq_sb[:], in_=wq_ext.rearrange("(a p) j -> p a j", p=128))
            for half in range(2):
                for cc in range(NCC):
                    eng = nc.sync if cc % 2 == 0 else nc.scalar
                    eng.dma_start(
                        out=xT[:, cc, half * 1024:(half + 1) * 1024],
                        in_=x_ext[cc * 128:(cc + 1) * 128, half * 1024:(half + 1) * 1024],
                    )
            nc.gpsimd.dma_start(out=wo_sb[:], in_=wo_ext.rearrange("(a p) j -> p a j", p=128))
            nc.gpsimd.dma_start(out=cosT[:], in_=cosT_d[:])
            nc.gpsimd.dma_start(out=sinTs[:], in_=sinTs_d[:])
            nc.gpsimd.dma_start(out=trimask[:], in_=trimask_d[:])
            nc.gpsimd.dma_start(out=blkones[:], in_=blkones_d[:])
            nc.gpsimd.dma_start(out=ident[:], in_=ident_d[:])
            nc.gpsimd.memset(eps_sb[:], EPS)
            nc.gpsimd.memset(v_sb[:, :, 64:128], 1.0)
            nc.gpsimd.memset(v_sb[:, :, 192:256], 1.0)

            with (
                tc.tile_pool(name="u_psum", bufs=1, space="PSUM") as up,
                tc.tile_pool(name="u_sbuf", bufs=3) as bs,
            ):
                qT_raw = bs.tile([128, 4, N], BF16, name="qT_raw", bufs=1)
                kT_raw = bs.tile([128, N], BF16, name="kT_raw", bufs=1)
                vT = bs.tile([128, N], BF16, name="vT", tag="vt", bufs=1)

                # ---- phase B pieces: projections ----
                def _proj(w_sb, wslice, dst, qp, tagsel):
                    # cc-outer / h-inner: one LDWEIGHTS serves two matmuls
                    pp = up.tile([128, 2, 512], F32, tag=tagsel, bufs=2,
                                 name=f"pp{qp}")
                    for cc in range(NCC):
                        for h in range(2):
                            qt = 2 * qp + h
                            nc.tensor.matmul(
                                pp[:, h, :], w_sb[:, cc, wslice],
                                xT[:, cc, qt * 512:(qt + 1) * 512],
                                start=(cc == 0), stop=(cc == NCC - 1),
                                skip_group_check=True,
                            )
                    nc.vector.tensor_copy(
                        dst[:, qp * 1024:(qp + 1) * 1024],
                        pp.rearrange("p a b -> p (a b)"),
                    )

                def emit_qproj(m):
                    for qp in range(NQT // 2):
                        _proj(wq_sb, slice(m * 128, (m + 1) * 128),
                              qT_raw[:, m, :], qp,
                              "mm" if (m * 2 + qp) % 2 == 0 else "o")

                def emit_kproj():
                    for qp in range(NQT // 2):
                        _proj(wk_sb, slice(0, DKV), kT_raw[:], qp,
                              "mm" if qp % 2 == 0 else "o")

                def emit_vproj():
                    for qp in range(NQT // 2):
                        _proj(wv_sb, slice(0, DKV), vT[:], qp,
                              "mm" if qp % 2 == 0 else "o")
                    for tcix in range(NTC):
                        pv = up.tile([128, 128], BF16,
                                     tag=("mm" if tcix % 2 == 0 else "o"),
                                     bufs=2, name=f"pvt{tcix}")
                        nc.tensor.transpose(pv[:], vT[:, tcix * 128:(tcix + 1) * 128], ident[:])
                        nc.vector.tensor_copy(v_sb[:, tcix, 0:64], pv[:, 0:64])
                        nc.vector.tensor_copy(v_sb[:, tcix, 128:192], pv[:, 64:128])

                # ---- phase C pieces: rms-norm + rope, chunk ci in 0..4 ----
                def emit_rope(ci):
                    if ci < 4:
                        src = dst = qT_raw[:, ci, :]
                    else:
                        src, dst = kT_raw[:], kTdA[:]
                    sq = bs.tile([128, N], BF16, tag="sq", bufs=2, name=f"sq{ci}")
                    nc.vector.tensor_mul(sq[:], src, src)
                    # block-diag ones reduce: mean-of-squares lands pre-broadcast
                    # across each head's 64 partitions; rsqrt via ln/exp.
                    lnv = bs.tile([128, N], F32, tag="lnv", bufs=1, name=f"lnv{ci}")
                    for qp in range(NQT // 2):
                        msp = up.tile([128, 2, 512], F32, tag="o", bufs=2, name=f"msp{ci}{qp}")
                        for h in range(2):
                            qt = 2 * qp + h
                            nc.tensor.matmul(
                                msp[:, h, :], blkones[:],
                                sq[:, qt * 512:(qt + 1) * 512], start=True, stop=True,
                            )
                        nc.scalar.activation(
                            lnv[:, qp * 1024:(qp + 1) * 1024],
                            msp.rearrange("p a b -> p (a b)"),
                            AF.Ln, bias=eps_sb[:], scale=1.0 / D,
                        )
                    rr2 = bs.tile([128, N], BF16, tag="rr2", bufs=1, name=f"rr2{ci}")
                    nc.scalar.activation(rr2[:], lnv[:], AF.Exp, scale=-0.5)
                    rot = bs.tile([128, N], BF16, tag="rot", bufs=2, name=f"rot{ci}")
                    nc.vector.tensor_copy(rot[0:32, :], src[32:64, :])
                    nc.vector.tensor_copy(rot[32:64, :], src[0:32, :])
                    nc.vector.tensor_copy(rot[64:96, :], src[96:128, :])
                    nc.vector.tensor_copy(rot[96:128, :], src[64:96, :])
                    t1 = bs.tile([128, N], BF16, tag="t1", bufs=2, name=f"t1{ci}")
                    nc.vector.tensor_mul(t1[:], src, cosT[:])
                    nc.vector.tensor_mul(rot[:], rot[:], sinTs[:])
                    nc.vector.tensor_add(t1[:], t1[:], rot[:])
                    nc.vector.tensor_mul(dst[:], t1[:], rr2[:])

                def emit_kdup():
                    # kTdA currently holds full kT (A rows 0:64, B 64:128)
                    nc.vector.tensor_copy(kTdB[0:64, :], kTdA[64:128, :])
                    nc.vector.tensor_copy(kTdB[64:128, :], kTdA[64:128, :])
                    nc.vector.tensor_copy(kTdA[64:128, :], kTdA[0:64, :])

                # ---- phase D: attention + partial out-proj + ReduceScatter ----
                qTf = qT_raw
                JL = NQT - 1
                # j<3: ReduceScatter per half-tile (2 token-chunks);
                # j=3 (the tail): per token-chunk so the last RS is small.
                rs_ins = [[dram.tile([2, 256, 512], BF16, name=f"rs_in{j}_{h}")
                           for h in range(2)] for j in range(JL)]
                rs_outs = [[dram.tile([256, 512], BF16, name=f"rs_out{j}_{h}")
                            for h in range(2)] for j in range(JL)]
                rs3_in = dram.tile([2, 512, 512], BF16, name="rs3_in")
                rs3_out = dram.tile([512, 512], BF16, name="rs3_out")

                def emit_outproj(jo, tts):
                    for tt in tts:
                        tcix = jo * 4 + tt
                        po = up.tile([128, 2, 512], F32, tag="o", bufs=2, name=f"po{jo}_{tt}")
                        for m4 in range(4):
                            for h in range(2):
                                nc.tensor.matmul(
                                    po[:, h, :], oT[:, m4, tcix * 128:(tcix + 1) * 128],
                                    wo_sb[:, m4, h * 512:(h + 1) * 512],
                                    start=(m4 == 0), stop=(m4 == 3),
                                    skip_group_check=True,
                                )
                        ev = bs.tile([128, 2, 512], BF16, tag="ev", bufs=2, name=f"ev{jo}_{tt}")
                        nc.vector.tensor_copy(ev[:], po[:])
                        for r in range(2):
                            if jo == JL:
                                nc.sync.dma_start(
                                    out=rs3_in[r, tt * 128:(tt + 1) * 128, :],
                                    in_=ev[:, r, :],
                                )
                            else:
                                nc.sync.dma_start(
                                    out=rs_ins[jo][tt // 2][r, (tt % 2) * 128:(tt % 2) * 128 + 128, :],
                                    in_=ev[:, r, :],
                                )

                def emit_rs(jo, h):
                    nc.gpsimd.collective_compute(
                        "ReduceScatter",
                        mybir.AluOpType.add,
                        replica_groups=[[0, 1], [2, 3], [4, 5], [6, 7]],
                        ins=[rs_ins[jo][h].opt()],
                        outs=[rs_outs[jo][h].opt()],
                    )
                    nc.sync.dma_start(
                        out=out_ext[jo * 512 + h * 256:jo * 512 + h * 256 + 256, :],
                        in_=rs_outs[jo][h][:],
                    )

                def emit_rs3():
                    nc.gpsimd.collective_compute(
                        "ReduceScatter",
                        mybir.AluOpType.add,
                        replica_groups=[[0, 1], [2, 3], [4, 5], [6, 7]],
                        ins=[rs3_in.opt()],
                        outs=[rs3_out.opt()],
                    )
                    nc.sync.dma_start(
                        out=out_ext[JL * 512:(JL + 1) * 512, :], in_=rs3_out[:]
                    )

                def emit_attn(j, m):
                    kT = kTdA if m < 2 else kTdB
                    vslot = 0 if m < 2 else 128
                    oab = up.tile([128, 2, 512], F32, tag="o", bufs=2, name=f"oab{j}{m}")
                    nkc = 4 * (j + 1)

                    def emit_scores(kc):
                        i = kc - 4 * j
                        off = max(0, 128 * i)
                        w = 512 - off
                        q0 = 512 * j + off
                        sAB = up.tile([128, 2, 512], F32, tag="mm", bufs=2,
                                      name=f"sAB{kc}")
                        nc.tensor.matmul(
                            sAB[:, 0, 0:w], kT[0:64, kc * 128:(kc + 1) * 128],
                            qTf[0:64, m, q0:q0 + w], start=True, stop=True,
                            tile_position=(0, 0),
                        )
                        nc.tensor.matmul(
                            sAB[:, 1, 0:w], kT[64:128, kc * 128:(kc + 1) * 128],
                            qTf[64:128, m, q0:q0 + w], start=True, stop=True,
                            tile_position=(64, 0),
                        )
                        pAB = bs.tile([128, 2, 512], BF16, tag="pAB", bufs=6,
                                      name=f"pAB{kc}")
                        nc.scalar.activation(
                            pAB[:, :, 0:w], sAB[:, :, 0:w], AF.Exp, scale=EXP_SCALE
                        )
                        if i >= 0:
                            nc.vector.tensor_mul(
                                pAB[:, :, 0:128], pAB[:, :, 0:128],
                                trimask.rearrange("p (a b) -> p a b", a=1)
                                .broadcast_to([128, 2, 128]),
                            )
                        return pAB

                    def emit_pv(kc, pAB):
                        i = kc - 4 * j
                        off = max(0, 128 * i)
                        w = 512 - off
                        for g in range(2):
                            nc.tensor.matmul(
                                oab[:, g, off:512], v_sb[:, kc, vslot:vslot + 128],
                                pAB[:, g, 0:w],
                                start=(kc == 0), stop=(kc == nkc - 1),
                                skip_group_check=True,
                            )

                    staged = []
                    for kc in range(nkc):
                        staged.append((kc, emit_scores(kc)))
                        if len(staged) == 2:
                            for kcx, px in staged:
                                emit_pv(kcx, px)
                            staged = []
                    for kcx, px in staged:
                        emit_pv(kcx, px)

                    # softmax normalization: rows 64:128 hold rowsum pre-bcast.
                    # reciprocal_approx_fast must run on SBUF (custom-DVE op
                    # misbehaves on PSUM inputs on HW), so evict the sum first.
                    ssum = bs.tile([64, 1024], F32, tag="ssum", bufs=2, name=f"ssum{j}{m}")
                    nc.vector.tensor_copy(ssum[:], oab[64:128, :, :])
                    rrf = bs.tile([64, 1024], F32, tag="rrf", bufs=2, name=f"rrf{j}{m}")
                    nc.vector.reciprocal_approx_fast(rrf[:], ssum[:])
                    nc.vector.tensor_mul(
                        oT[0:64, m, 512 * j:512 * (j + 1)], oab[0:64, 0, :], rrf[:, 0:512]
                    )
                    nc.vector.tensor_mul(
                        oT[64:128, m, 512 * j:512 * (j + 1)], oab[0:64, 1, :], rrf[:, 512:1024]
                    )

                # ---- interleaved emission ----
                # B/C chunks feed the PE ahead of DVE rope; j=0 attention
                # slots between Q-chunk projections; out-proj for tile j-1
                # spreads one token-chunk per m of tile j.
                emit_kproj()
                emit_vproj()
                emit_rope(4)
                emit_kdup()
                emit_qproj(0)
                emit_rope(0)
                emit_qproj(1)
                emit_rope(1)
                emit_attn(0, 0)
                emit_qproj(2)
                emit_rope(2)
                emit_attn(0, 1)
                emit_qproj(3)
                emit_rope(3)
                emit_attn(0, 2)
                emit_attn(0, 3)
                for j in range(1, NQT):
                    for m in range(4):
                        emit_attn(j, m)
                        if m <= 1:
                            emit_outproj(j - 1, [2 * m, 2 * m + 1])
                            emit_rs(j - 1, m)
                for tt in range(4):
                    emit_outproj(JL, [tt])
                emit_rs3()

    nc.finalize()
    return nc


_NC_CACHE = None


def _get_nc():
    global _NC_CACHE
    if _NC_CACHE is None:
        _NC_CACHE = build()
    return _NC_CACHE


def _make_in_maps(inputs):
    x = np.asarray(inputs["x"], dtype=np.float32)
    wq = np.asarray(inputs["wq"], dtype=np.float32)
    wk = np.asarray(inputs["wk"], dtype=np.float32)
    wv = np.asarray(inputs["wv"], dtype=np.float32)
    wo = np.asarray(inputs["wo"], dtype=np.float32)
    bf = ml_dtypes.bfloat16
    in_maps = []
    for i in range(8):
        b, p = i // 2, i % 2
        in_maps.append({
            "x": np.ascontiguousarray(x[b].T.astype(bf)),
            "wq": np.ascontiguousarray(wq[:, p * DQ:(p + 1) * DQ].astype(bf)),
            "wk": np.ascontiguousarray(wk[:, p * DKV:(p + 1) * DKV].astype(bf)),
            "wv": np.ascontiguousarray(wv[:, p * DKV:(p + 1) * DKV].astype(bf)),
            "wo": np.ascontiguousarray(wo[p * DQ:(p + 1) * DQ, :].astype(bf)),
        })
    return in_maps


def kernel(x, wq, wk, wv, wo):
    x = np.asarray(x, dtype=np.float32)
    B = x.shape[0]
    nc = _get_nc()
    in_maps = _make_in_maps({"x": x, "wq": wq, "wk": wk, "wv": wv, "wo": wo})
    res = run_bass_kernel_spmd(nc, in_maps, core_ids=list(range(8)))
    out = np.empty((B, N, C), dtype=np.float32)
    for b in range(B):
        out[b, :, 0:DQ] = res.results[2 * b]["out"].astype(np.float32)
        out[b, :, DQ:C] = res.results[2 * b + 1]["out"].astype(np.float32)
    return out


if __name__ == "__main__":
    rng = np.random.default_rng(0)
    ins = {
        "x": rng.standard_normal((4, N, C), dtype=np.float32),
        "wq": (rng.standard_normal((C, C), dtype=np.float32) * 0.02),
        "wk": (rng.standard_normal((C, 256), dtype=np.float32) * 0.02),
        "wv": (rng.standard_normal((C, 256), dtype=np.float32) * 0.02),
        "wo": (rng.standard_normal((C, C), dtype=np.float32) * 0.02),
    }
    y = kernel(**ins)
    print("out", y.shape, y.dtype, np.abs(y).mean())


# revision 35
# speedup vs baseline: 1.0405x; 1.0405x over previous
"""Distributed Trainium2 Bass kernel for causal GQA attention block.

Problem (hardcoded): x [4, 2048, 1024] f32; wq [1024, 1024]; wk/wv [1024, 256];
wo [1024, 1024]. 16 q-heads, 4 kv-heads, head_dim 64, rms-norm on q/k (no
weight), rope (base 10000), q gain 1.5, causal SDPA, out-proj.

Sharding over 8 cores: core i -> batch b = i//2, head-half p = i%2
(q-heads 8p..8p+7, kv-heads 2p, 2p+1 -- KV groups intact). Each core computes
its 8 heads' attention output O^T (feature-major), then a PARTIAL out-proj
over its own 512 contraction dims against wo[own rows, all 1024 cols]; a
pairwise ReduceScatter(add) finishes the projection and leaves each core
with its disjoint 512-column output slice.

On-chip layouts are feature-major ("transposed"): X^T, Q^T, K^T so the PE
contracts over partitions; V is token-major with a ones column appended so the
PV matmul also produces softmax row-sums (normalization happens on O^T).
"""
import sys

sys.path.insert(0, "/opt/trn_rl_repo")

import numpy as np
import ml_dtypes

import concourse.bacc as bacc
import concourse.mybir as mybir
import concourse.tile as tile
from concourse.bass_utils import run_bass_kernel_spmd

F32 = mybir.dt.float32
BF16 = mybir.dt.bfloat16
AF = mybir.ActivationFunctionType

N = 2048          # tokens
C = 1024          # model dim
DQ = 512          # local q out-features (8 heads x 64)
DKV = 128         # local kv out-features (2 kv heads x 64)
D = 64            # head dim
NCC = C // 128    # 8 contraction chunks
NQT = 4           # q tiles of 512
NTC = N // 128    # 16 token chunks
QK_GAIN = 1.5
ROPE_BASE = 10000.0
EXP_SCALE = QK_GAIN / np.sqrt(D).item()  # folded gain * 1/sqrt(D) = 0.1875
EPS = float(np.finfo(np.float32).eps)


def _host_tables():
    inv_freq = (1.0 / (ROPE_BASE ** (np.arange(0, D, 2, dtype=np.float64) / D)))  # [32]
    t = np.arange(N, dtype=np.float64)
    ang = np.outer(inv_freq, t)  # [32, N]
    cos32 = np.cos(ang)
    sin32 = np.sin(ang)
    cosT = np.tile(cos32, (4, 1)).astype(np.float32)  # [128, N]
    sinTs = np.concatenate([-sin32, sin32, -sin32, sin32], axis=0).astype(np.float32)
    q = np.arange(128)
    trimask = (q[None, :] >= q[:, None]).astype(np.float32)  # keep q >= k
    # block-diagonal ones: reduce-over-64-feats that lands pre-broadcast
    blkones = np.zeros((128, 128), np.float32)
    blkones[0:64, 0:64] = 1.0
    blkones[64:128, 64:128] = 1.0
    ident = np.eye(128, dtype=np.float32)
    bf = ml_dtypes.bfloat16
    return {
        "cosT": cosT.astype(bf),
        "sinTs": sinTs.astype(bf),
        "trimask": trimask.astype(bf),
        "blkones": blkones.astype(bf),
        "ident": ident.astype(bf),
    }


def build():
    nc = bacc.Bacc(None, target_bir_lowering=False, num_devices=8)

    # x arrives host-pre-transposed: [C, N] feature-major
    x_ext = nc.declare_dram_parameter("x", [C, N], BF16, isOutput=False)
    wq_ext = nc.declare_dram_parameter("wq", [C, DQ], BF16, isOutput=False)
    wk_ext = nc.declare_dram_parameter("wk", [C, DKV], BF16, isOutput=False)
    wv_ext = nc.declare_dram_parameter("wv", [C, DKV], BF16, isOutput=False)
    # wo here is the core's OWN 512 contraction rows x all 1024 out-features.
    wo_ext = nc.declare_dram_parameter("wo", [DQ, C], BF16, isOutput=False)
    out_ext = nc.declare_dram_parameter("out", [N, DQ], BF16, isOutput=True)

    tabs = _host_tables()
    cosT_d = nc.inline_tensor(tabs["cosT"], name="cosT_d")
    sinTs_d = nc.inline_tensor(tabs["sinTs"], name="sinTs_d")
    trimask_d = nc.inline_tensor(tabs["trimask"], name="trimask_d")
    blkones_d = nc.inline_tensor(tabs["blkones"], name="blkones_d")
    ident_d = nc.inline_tensor(tabs["ident"], name="ident_d")

    with tile.TileContext(nc) as tc:
        with (
            tc.tile_pool(name="dram", bufs=1, space="DRAM") as dram,
            tc.tile_pool(name="persist", bufs=1) as ps,
        ):
            # ---- persistent SBUF tensors ----
            xT = ps.tile([128, NCC, N], BF16, name="xT")          # X^T chunks
            wq_sb = ps.tile([128, NCC, DQ], BF16, name="wq_sb")
            wk_sb = ps.tile([128, NCC, DKV], BF16, name="wk_sb")
            wv_sb = ps.tile([128, NCC, DKV], BF16, name="wv_sb")
            wo_sb = ps.tile([128, 4, C], BF16, name="wo_sb")      # own rows x 1024
            cosT = ps.tile([128, N], BF16, name="cosT")
            sinTs = ps.tile([128, N], BF16, name="sinTs")
            trimask = ps.tile([128, 128], BF16, name="trimask")
            blkones = ps.tile([128, 128], BF16, name="blkones")
            ident = ps.tile([128, 128], BF16, name="ident")
            eps_sb = ps.tile([128, 1], F32, name="eps_sb")
            kTdA = ps.tile([128, N], BF16, name="kTdA")           # kv head A dup'd
            kTdB = ps.tile([128, N], BF16, name="kTdB")
            # [V_A(64)|1s(64)|V_B(64)|1s(64)] -> PV rows 64:128 = rowsum bcast
            v_sb = ps.tile([128, NTC, 256], BF16, name="v_sb")
            oT = ps.tile([128, 4, N], BF16, name="oT")            # own O^T (normed)

            # ---- phase A: stage inputs (host supplies bf16, x pre-transposed) ----
            # wk/wq first (first matmuls need them), x chunks split across queues
            nc.gpsimd.dma_start(out=wk_sb[:], in_=wk_ext.rearrange("(a p) j -> p a j", p=128))
            nc.gpsimd.dma_start(out=wv_sb[:], in_=wv_ext.rearrange("(a p) j -> p a j", p=128))
            nc.gpsimd.dma_start(out=wq_sb[:], in_=wq_ext.rearrange("(a p) j -> p a j", p=128))
            for half in range(2):
                for cc in range(NCC):
                    eng = nc.sync if cc % 2 == 0 else nc.scalar
                    eng.dma_start(
                        out=xT[:, cc, half * 1024:(half + 1) * 1024],
                        in_=x_ext[cc * 128:(cc + 1) * 128, half * 1024:(half + 1) * 1024],
                    )
            nc.gpsimd.dma_start(out=wo_sb[:], in_=wo_ext.rearrange("(a p) j -> p a j", p=128))
            nc.gpsimd.dma_start(out=cosT[:], in_=cosT_d[:])
            nc.gpsimd.dma_start(out=sinTs[:], in_=sinTs_d[:])
            nc.gpsimd.dma_start(out=trimask[:], in_=trimask_d[:])
            nc.gpsimd.dma_start(out=blkones[:], in_=blkones_d[:])
            nc.gpsimd.dma_start(out=ident[:], in_=ident_d[:])
            nc.gpsimd.memset(eps_sb[:], EPS)
            nc.gpsimd.memset(v_sb[:, :, 64:128], 1.0)
            nc.gpsimd.memset(v_sb[:, :, 192:256], 1.0)

            with (
                tc.tile_pool(name="u_psum", bufs=1, space="PSUM") as up,
                tc.tile_pool(name="u_sbuf", bufs=3) as bs,
            ):
                qT_raw = bs.tile([128, 4, N], BF16, name="qT_raw", bufs=1)
                kT_raw = bs.tile([128, N], BF16, name="kT_raw", bufs=1)
                vT = bs.tile([128, N], BF16, name="vT", tag="vt", bufs=1)

                # ---- phase B pieces: projections ----
                def _proj(w_sb, wslice, dst, qp, tagsel):
                    # cc-outer / h-inner: one LDWEIGHTS serves two matmuls
                    pp = up.tile([128, 2, 512], F32, tag=tagsel, bufs=2,
                                 name=f"pp{qp}")
                    for cc in range(NCC):
                        for h in range(2):
                            qt = 2 * qp + h
                            nc.tensor.matmul(
                                pp[:, h, :], w_sb[:, cc, wslice],
                                xT[:, cc, qt * 512:(qt + 1) * 512],
                                start=(cc == 0), stop=(cc == NCC - 1),
                                skip_group_check=True,
                            )
                    nc.vector.tensor_copy(
                        dst[:, qp * 1024:(qp + 1) * 1024],
                        pp.rearrange("p a b -> p (a b)"),
                    )

                def emit_qproj(m):
                    for qp in range(NQT // 2):
                        _proj(wq_sb, slice(m * 128, (m + 1) * 128),
                              qT_raw[:, m, :], qp,
                              "mm" if (m * 2 + qp) % 2 == 0 else "o")

                def emit_kproj():
                    for qp in range(NQT // 2):
                        _proj(wk_sb, slice(0, DKV), kT_raw[:], qp,
                              "mm" if qp % 2 == 0 else "o")

                def emit_vproj():
                    for qp in range(NQT // 2):
                        _proj(wv_sb, slice(0, DKV), vT[:], qp,
                              "mm" if qp % 2 == 0 else "o")
                    for tcix in range(NTC):
                        pv = up.tile([128, 128], BF16,
                                     tag=("mm" if tcix % 2 == 0 else "o"),
                                     bufs=2, name=f"pvt{tcix}")
                        nc.tensor.transpose(pv[:], vT[:, tcix * 128:(tcix + 1) * 128], ident[:])
                        nc.vector.tensor_copy(v_sb[:, tcix, 0:64], pv[:, 0:64])
                        nc.vector.tensor_copy(v_sb[:, tcix, 128:192], pv[:, 64:128])

                # ---- phase C pieces: rms-norm + rope, chunk ci in 0..4 ----
                def emit_rope(ci):
                    if ci < 4:
                        src = dst = qT_raw[:, ci, :]
                    else:
                        src, dst = kT_raw[:], kTdA[:]
                    sq = bs.tile([128, N], BF16, tag="sq", bufs=2, name=f"sq{ci}")
                    nc.vector.tensor_mul(sq[:], src, src)
                    # block-diag ones reduce: mean-of-squares lands pre-broadcast
                    # across each head's 64 partitions; rsqrt via ln/exp.
                    lnv = bs.tile([128, N], F32, tag="lnv", bufs=1, name=f"lnv{ci}")
                    for qp in range(NQT // 2):
                        msp = up.tile([128, 2, 512], F32, tag="o", bufs=2, name=f"msp{ci}{qp}")
                        for h in range(2):
                            qt = 2 * qp + h
                            nc.tensor.matmul(
                                msp[:, h, :], blkones[:],
                                sq[:, qt * 512:(qt + 1) * 512], start=True, stop=True,
                            )
                        nc.scalar.activation(
                            lnv[:, qp * 1024:(qp + 1) * 1024],
                            msp.rearrange("p a b -> p (a b)"),
                            AF.Ln, bias=eps_sb[:], scale=1.0 / D,
                        )
                    rr2 = bs.tile([128, N], BF16, tag="rr2", bufs=1, name=f"rr2{ci}")
                    nc.scalar.activation(rr2[:], lnv[:], AF.Exp, scale=-0.5)
                    rot = bs.tile([128, N], BF16, tag="rot", bufs=2, name=f"rot{ci}")
                    nc.vector.tensor_copy(rot[0:32, :], src[32:64, :])
                    nc.vector.tensor_copy(rot[32:64, :], src[0:32, :])
                    nc.vector.tensor_copy(rot[64:96, :], src[96:128, :])
                    nc.vector.tensor_copy(rot[96:128, :], src[64:96, :])
                    t1 = bs.tile([128, N], BF16, tag="t1", bufs=2, name=f"t1{ci}")
                    nc.vector.tensor_mul(t1[:], src, cosT[:])
                    nc.vector.tensor_mul(rot[:], rot[:], sinTs[:])
                    nc.vector.tensor_add(t1[:], t1[:], rot[:])
                    nc.vector.tensor_mul(dst[:], t1[:], rr2[:])

                def emit_kdup():
                    # kTdA currently holds full kT (A rows 0:64, B 64:128)
                    nc.vector.tensor_copy(kTdB[0:64, :], kTdA[64:128, :])
                    nc.vector.tensor_copy(kTdB[64:128, :], kTdA[64:128, :])
                    nc.vector.tensor_copy(kTdA[64:128, :], kTdA[0:64, :])

                # ---- phase D: attention + partial out-proj + ReduceScatter ----
                qTf = qT_raw
                JL = NQT - 1
                # j<3: ReduceScatter per half-tile (2 token-chunks);
                # j=3 (the tail): per token-chunk so the last RS is small.
                rs_ins = [[dram.tile([2, 256, 512], BF16, name=f"rs_in{j}_{h}")
                           for h in range(2)] for j in range(JL)]
                rs_outs = [[dram.tile([256, 512], BF16, name=f"rs_out{j}_{h}")
                            for h in range(2)] for j in range(JL)]
                rs3_in = dram.tile([2, 512, 512], BF16, name="rs3_in")
                rs3_out = dram.tile([512, 512], BF16, name="rs3_out")

                def emit_outproj(jo, tts):
                    for tt in tts:
                        tcix = jo * 4 + tt
                        po = up.tile([128, 2, 512], F32, tag="o", bufs=2, name=f"po{jo}_{tt}")
                        for m4 in range(4):
                            for h in range(2):
                                nc.tensor.matmul(
                                    po[:, h, :], oT[:, m4, tcix * 128:(tcix + 1) * 128],
                                    wo_sb[:, m4, h * 512:(h + 1) * 512],
                                    start=(m4 == 0), stop=(m4 == 3),
                                    skip_group_check=True,
                                )
                        ev = bs.tile([128, 2, 512], BF16, tag="ev", bufs=2, name=f"ev{jo}_{tt}")
                        nc.vector.tensor_copy(ev[:], po[:])
                        for r in range(2):
                            if jo == JL:
                                nc.sync.dma_start(
                                    out=rs3_in[r, tt * 128:(tt + 1) * 128, :],
                                    in_=ev[:, r, :],
                                )
                            else:
                                nc.sync.dma_start(
                                    out=rs_ins[jo][tt // 2][r, (tt % 2) * 128:(tt % 2) * 128 + 128, :],
                                    in_=ev[:, r, :],
                                )

                def emit_rs(jo, h):
                    nc.gpsimd.collective_compute(
                        "ReduceScatter",
                        mybir.AluOpType.add,
                        replica_groups=[[0, 1], [2, 3], [4, 5], [6, 7]],
                        ins=[rs_ins[jo][h].opt()],
                        outs=[rs_outs[jo][h].opt()],
                    )
                    nc.sync.dma_start(
                        out=out_ext[jo * 512 + h * 256:jo * 512 + h * 256 + 256, :],
                        in_=rs_outs[jo][h][:],
                    )

                def emit_rs3():
                    nc.gpsimd.collective_compute(
                        "ReduceScatter",
                        mybir.AluOpType.add,
                        replica_groups=[[0, 1], [2, 3], [4, 5], [6, 7]],
                        ins=[rs3_in.opt()],
                        outs=[rs3_out.opt()],
                    )
                    nc.sync.dma_start(
                        out=out_ext[JL * 512:(JL + 1) * 512, :], in_=rs3_out[:]
                    )

                def emit_attn(j, m):
                    kT = kTdA if m < 2 else kTdB
                    vslot = 0 if m < 2 else 128
                    oab = up.tile([128, 2, 512], F32, tag="o", bufs=2, name=f"oab{j}{m}")
                    nkc = 4 * (j + 1)

                    def emit_scores(kc):
                        i = kc - 4 * j
                        off = max(0, 128 * i)
                        w = 512 - off
                        q0 = 512 * j + off
                        sAB = up.tile([128, 2, 512], F32, tag="mm", bufs=2,
                                      name=f"sAB{kc}")
                        nc.tensor.matmul(
                            sAB[:, 0, 0:w], kT[0:64, kc * 128:(kc + 1) * 128],
                            qTf[0:64, m, q0:q0 + w], start=True, stop=True,
                            tile_position=(0, 0),
                        )
                        nc.tensor.matmul(
                            sAB[:, 1, 0:w], kT[64:128, kc * 128:(kc + 1) * 128],
                            qTf[64:128, m, q0:q0 + w], start=True, stop=True,
                            tile_position=(64, 0),
                        )
                        pAB = bs.tile([128, 2, 512], BF16, tag="pAB", bufs=6,
                                      name=f"pAB{kc}")
                        nc.scalar.activation(
                            pAB[:, :, 0:w], sAB[:, :, 0:w], AF.Exp, scale=EXP_SCALE
                        )
                        if i >= 0:
                            nc.vector.tensor_mul(
                                pAB[:, :, 0:128], pAB[:, :, 0:128],
                                trimask.rearrange("p (a b) -> p a b", a=1)
                                .broadcast_to([128, 2, 128]),
                            )
                        return pAB

                    def emit_pv(kc, pAB):
                        i = kc - 4 * j
                        off = max(0, 128 * i)
                        w = 512 - off
                        for g in range(2):
                            nc.tensor.matmul(
                                oab[:, g, off:512], v_sb[:, kc, vslot:vslot + 128],
                                pAB[:, g, 0:w],
                                start=(kc == 0), stop=(kc == nkc - 1),
                                skip_group_check=True,
                            )

                    staged = []
                    for kc in range(nkc):
                        staged.append((kc, emit_scores(kc)))
                        if len(staged) == 2:
                            for kcx, px in staged:
                                emit_pv(kcx, px)
                            staged = []
                    for kcx, px in staged:
                        emit_pv(kcx, px)

                    # softmax normalization: rows 64:128 hold rowsum pre-bcast.
                    # reciprocal_approx_fast must run on SBUF (custom-DVE op
                    # misbehaves on PSUM inputs on HW), so evict the sum first.
                    ssum = bs.tile([64, 1024], F32, tag="ssum", bufs=2, name=f"ssum{j}{m}")
                    nc.vector.tensor_copy(ssum[:], oab[64:128, :, :])
                    rrf = bs.tile([64, 1024], F32, tag="rrf", bufs=2, name=f"rrf{j}{m}")
                    nc.vector.reciprocal_approx_fast(rrf[:], ssum[:])
                    nc.vector.tensor_mul(
                        oT[0:64, m, 512 * j:512 * (j + 1)], oab[0:64, 0, :], rrf[:, 0:512]
                    )
                    nc.vector.tensor_mul(
                        oT[64:128, m, 512 * j:512 * (j + 1)], oab[0:64, 1, :], rrf[:, 512:1024]
                    )

                # ---- interleaved emission ----
                # B/C chunks feed the PE ahead of DVE rope; j=0 attention
                # slots between Q-chunk projections; out-proj for tile j-1
                # spreads one token-chunk per m of tile j.
                emit_kproj()
                emit_vproj()
                emit_rope(4)
                emit_kdup()
                emit_qproj(0)
                emit_rope(0)
                emit_qproj(1)
                emit_rope(1)
                emit_attn(0, 0)
                emit_qproj(2)
                emit_rope(2)
                emit_attn(0, 1)
                emit_qproj(3)
                emit_rope(3)
                emit_attn(0, 2)
                emit_attn(0, 3)
                for j in range(1, NQT):
                    for m in range(4):
                        emit_attn(j, m)
                        emit_outproj(j - 1, [m])
                        if m == 1:
                            emit_rs(j - 1, 0)
                        elif m == 3:
                            emit_rs(j - 1, 1)
                for tt in range(4):
                    emit_outproj(JL, [tt])
                emit_rs3()

    nc.finalize()
    return nc


_NC_CACHE = None


def _get_nc():
    global _NC_CACHE
    if _NC_CACHE is None:
        _NC_CACHE = build()
    return _NC_CACHE


def _make_in_maps(inputs):
    x = np.asarray(inputs["x"], dtype=np.float32)
    wq = np.asarray(inputs["wq"], dtype=np.float32)
    wk = np.asarray(inputs["wk"], dtype=np.float32)
    wv = np.asarray(inputs["wv"], dtype=np.float32)
    wo = np.asarray(inputs["wo"], dtype=np.float32)
    bf = ml_dtypes.bfloat16
    in_maps = []
    for i in range(8):
        b, p = i // 2, i % 2
        in_maps.append({
            "x": np.ascontiguousarray(x[b].T.astype(bf)),
            "wq": np.ascontiguousarray(wq[:, p * DQ:(p + 1) * DQ].astype(bf)),
            "wk": np.ascontiguousarray(wk[:, p * DKV:(p + 1) * DKV].astype(bf)),
            "wv": np.ascontiguousarray(wv[:, p * DKV:(p + 1) * DKV].astype(bf)),
            "wo": np.ascontiguousarray(wo[p * DQ:(p + 1) * DQ, :].astype(bf)),
        })
    return in_maps


def kernel(x, wq, wk, wv, wo):
    x = np.asarray(x, dtype=np.float32)
    B = x.shape[0]
    nc = _get_nc()
    in_maps = _make_in_maps({"x": x, "wq": wq, "wk": wk, "wv": wv, "wo": wo})
    res = run_bass_kernel_spmd(nc, in_maps, core_ids=list(range(8)))
    out = np.empty((B, N, C), dtype=np.float32)
    for b in range(B):
        out[b, :, 0:DQ] = res.results[2 * b]["out"].astype(np.float32)
        out[b, :, DQ:C] = res.results[2 * b + 1]["out"].astype(np.float32)
    return out


if __name__ == "__main__":
    rng = np.random.default_rng(0)
    ins = {
        "x": rng.standard_normal((4, N, C), dtype=np.float32),
        "wq": (rng.standard_normal((C, C), dtype=np.float32) * 0.02),
        "wk": (rng.standard_normal((C, 256), dtype=np.float32) * 0.02),
        "wv": (rng.standard_normal((C, 256), dtype=np.float32) * 0.02),
        "wo": (rng.standard_normal((C, C), dtype=np.float32) * 0.02),
    }
    y = kernel(**ins)
    print("out", y.shape, y.dtype, np.abs(y).mean())
